# revision 7
# baseline (speedup 1.0000x reference)
"""Trainium2 Bass kernel for nn_CascadingSystem (confidence-gated 2-expert blend).

Computation (reference):
    xf = x.reshape(256, 150528)
    t_out = xf @ W1 + b1            # [256, 2]
    f_out = xf @ W2 + b2            # [256, 2]
    conf  = max(softmax(t_out, 1), 1)
    out   = where(conf > 0.95, t_out, 0.7*t_out + 0.3*f_out)

Strategy (memory-bound; the 154 MB read of x dominates):
  - Shard the feature dim D=150528 across 8 cores (18816 each). Every core
    streams its d-slice of ALL 256 samples once from HBM and computes the
    partial [4, 256] logits (4 = W1c0, W1c1, W2c0, W2c1) on the tensor
    engine: 147 accumulating matmuls, stationary W chunk [128, 4], moving
    x chunk [128, 256].
  - The host pre-packs x into the PE-ready layout (d on partitions), so
    device DMAs are fully contiguous and run at the HBM roofline.
  - Raw Bass (no TileContext): explicit per-DMA semaphores; the PE chases
    the 21 input DMAs chunk-by-chunk. Avoids Tile's multi-wait drain
    (walrus allows very few embedded sync waits per instruction) and its
    ~10us kernel-tail barrier.
  - Host sums the 8 partial [4, 256] tensors and applies the tiny
    bias/softmax/threshold/blend epilogue on [256, 4] floats.
"""

from contextlib import ExitStack

import numpy as np

import concourse.bass as bass
import concourse.mybir as mybir
from concourse.bass_utils import run_bass_kernel_spmd

NCORES = 8
B = 256            # batch (streamed as matmul moving dim)
D = 150528         # 3*224*224
DS = D // NCORES   # 18816 features per core
P = 128            # partitions / contraction tile
J = DS // P        # 147 matmul chunks per core
NDMA = 21          # x DMA chunks per core
JPD = -(-J // NDMA)  # matmul chunks per DMA chunk
CHUNK = JPD * B    # x columns per full DMA chunk
WCOLS = 4 * J      # 588 weight columns (4 per chunk)
TOT = WCOLS + J * B  # total columns per core
THRESHOLD = 0.95

_CACHE = {}


def _build():
    nc = bass.Bass()
    xw_in = nc.declare_dram_parameter("xw", [P, TOT], mybir.dt.float32, isOutput=False)
    out = nc.declare_dram_parameter("partial", [4, B], mybir.dt.float32, isOutput=True)

    with ExitStack() as ctx:
        # chunk 0 carries W (WCOLS cols) + first x chunk
        t0 = ctx.enter_context(
            nc.sbuf_tensor("t0", [P, WCOLS + CHUNK], mybir.dt.float32)
        )
        xts = [t0]
        for d in range(1, NDMA):
            ncols = (min((d + 1) * JPD, J) - d * JPD) * B
            xts.append(
                ctx.enter_context(nc.sbuf_tensor(f"xt{d}", [P, ncols], mybir.dt.float32))
            )
        out_sb = ctx.enter_context(nc.sbuf_tensor("out_sb", [4, B], mybir.dt.float32))
        acc = ctx.enter_context(nc.psum_tensor("acc", [4, B], mybir.dt.float32))

        dsems = [ctx.enter_context(nc.semaphore(f"d{d}")) for d in range(NDMA)]
        pe_sem = ctx.enter_context(nc.semaphore("pe"))
        dve_sem = ctx.enter_context(nc.semaphore("dve"))
        osem = ctx.enter_context(nc.semaphore("o"))
        all_sems = dsems + [pe_sem, dve_sem, osem]

        block = ctx.enter_context(nc.Block())

        @block.sync
        def _(sync):
            sync.dma_start(t0[:], xw_in[:, 0 : WCOLS + CHUNK]).then_inc(dsems[0], 16)
            for d in range(1, NDMA):
                ncols = xts[d].shape[1]
                sync.dma_start(
                    xts[d][:], xw_in[:, WCOLS + d * CHUNK : WCOLS + d * CHUNK + ncols]
                ).then_inc(dsems[d], 16)
            sync.wait_ge(dve_sem, 1)
            sync.dma_start(out[:], out_sb[:]).then_inc(osem, 16)

        @block.tensor
        def _(tensor):
            for j in range(J):
                d, jj = divmod(j, JPD)
                if jj == 0:
                    tensor.wait_ge(dsems[d], 16)
                xoff = (WCOLS if d == 0 else 0) + jj * B
                mm = tensor.matmul(
                    acc[:],
                    t0[:, 4 * j : 4 * j + 4],
                    xts[d][:, xoff : xoff + B],
                    start=(j == 0),
                    stop=(j == J - 1),
                )
            mm.then_inc(pe_sem, 1)

        @block.vector
        def _(vector):
            vector.wait_ge(pe_sem, 1)
            vector.tensor_copy(out_sb[:], acc[:]).then_inc(dve_sem, 1)

        @block.gpsimd
        def _(gpsimd):
            # reset all sems to 0 after everything finished so a cached
            # NEFF can be re-executed (sem state persists across runs)
            gpsimd.wait_ge(osem, 16)
            for s in all_sems:
                gpsimd.sem_clear(s)

    return nc


def _pack(x, W1, W2):
    xf = np.ascontiguousarray(x, dtype=np.float32).reshape(B, D)

    w4 = np.concatenate(
        [np.asarray(W1, np.float32), np.asarray(W2, np.float32)], axis=1
    )  # [D, 4]

    xw = np.empty((NCORES, P, TOT), dtype=np.float32)
    # wp[k, p, 4j + c] = w4[k*DS + j*P + p, c]
    xw[:, :, :WCOLS] = (
        w4.reshape(NCORES, J, P, 4).transpose(0, 2, 1, 3).reshape(NCORES, P, WCOLS)
    )
    # xp[k, p, j*B + b] = xf[b, k*DS + j*P + p]
    xw[:, :, WCOLS:] = xf.reshape(B, NCORES, J, P).transpose(1, 3, 2, 0).reshape(
        NCORES, P, J * B
    )
    return xw


def kernel(x, W1, b1, W2, b2, trace=False):
    if "nc" not in _CACHE:
        _CACHE["nc"] = _build()
    nc = _CACHE["nc"]

    xw = _pack(x, W1, W2)
    in_maps = [{"xw": xw[k]} for k in range(NCORES)]
    res = run_bass_kernel_spmd(nc, in_maps, list(range(NCORES)), trace=trace)
    _CACHE["last_results"] = res

    logits4 = np.zeros((4, B), dtype=np.float32)
    for k in range(NCORES):
        logits4 += res.results[k]["partial"]

    t_out = logits4[0:2].T + np.asarray(b1, np.float32)  # [256, 2]
    f_out = logits4[2:4].T + np.asarray(b2, np.float32)  # [256, 2]
    m = t_out.max(axis=1, keepdims=True)
    e = np.exp(t_out - m)
    conf = (e / e.sum(axis=1, keepdims=True)).max(axis=1)
    blended = 0.7 * t_out + 0.3 * f_out
    out = np.where((conf > THRESHOLD)[:, None], t_out, blended)
    return out.astype(np.float32)


# revision 9
# speedup vs baseline: 1.2846x; 1.2846x over previous
"""Trainium2 Bass kernel for nn_CascadingSystem (confidence-gated 2-expert blend).

Computation (reference):
    xf = x.reshape(256, 150528)
    t_out = xf @ W1 + b1            # [256, 2]
    f_out = xf @ W2 + b2            # [256, 2]
    conf  = max(softmax(t_out, 1), 1)
    out   = where(conf > 0.95, t_out, 0.7*t_out + 0.3*f_out)

Strategy (memory-bound; the 154 MB read of x dominates):
  - Shard the feature dim D=150528 across 8 cores (18816 each). Every core
    streams its d-slice of ALL 256 samples once from HBM and computes the
    partial [4, 256] logits (4 = W1c0, W1c1, W2c0, W2c1) on the tensor
    engine.
  - fp32 matmuls run at 4 cycles/row on TRN2 (LOW_HIGH double pass), which
    made the PE the bottleneck (67us busy).  Instead split every operand
    into fp16 hi + fp16 residual on the host (x = xh + xr, W = wh + wl,
    each half fp16 = same total HBM bytes as fp32) and compute
        logits = xh*wh + xr*wh + xh*wl     (xr*wl term ~2^-22, dropped)
    as ONE matmul per 128-feature chunk: stationary [wh|wl] = [128, 8],
    moving [xh|xr] = [128, 512] -> psum [8, 512] holds all three useful
    products (plus an unused xr*wl block).  1 cycle/row fp16 => PE ~32us,
    fully under the ~54us DMA roofline.  Max logit error ~2e-5 (the
    conf>0.95 gate's closest sample sits 7.2e-4 away - 100x margin).
  - The host pre-packs x into the PE-ready layout (feature dim on
    partitions), so device DMAs are fully contiguous.
  - Raw Bass (no TileContext): explicit per-DMA semaphores; the PE chases
    the 21 input DMAs chunk-by-chunk. Avoids Tile's multi-wait drain
    (walrus allows very few embedded sync waits per instruction) and its
    ~10us kernel-tail barrier.
  - Host sums the 8 partial [4, 256] tensors and applies the tiny
    bias/softmax/threshold/blend epilogue on [256, 4] floats.
"""

from contextlib import ExitStack

import numpy as np

import concourse.bass as bass
import concourse.mybir as mybir
from concourse.bass_utils import run_bass_kernel_spmd

NCORES = 8
B = 256            # batch
D = 150528         # 3*224*224
DS = D // NCORES   # 18816 features per core
P = 128            # partitions / contraction tile
J = DS // P        # 147 matmul chunks per core
MCOLS = 2 * B      # 512 moving columns per chunk (xh | xr)
NDMA = 21          # input DMA chunks per core
JPD = -(-J // NDMA)  # matmul chunks per DMA chunk
CHUNK = JPD * MCOLS  # x columns per full DMA chunk
WCOLS = 8 * J      # 1176 weight columns (wh|wl, 4 each, per chunk)
TOT = WCOLS + J * MCOLS
THRESHOLD = 0.95

_CACHE = {}


def _build():
    nc = bass.Bass()
    xw_in = nc.declare_dram_parameter("xw", [P, TOT], mybir.dt.float16, isOutput=False)
    out = nc.declare_dram_parameter(
        "partial", [8, MCOLS], mybir.dt.float32, isOutput=True
    )

    with ExitStack() as ctx:
        # chunk 0 carries W (WCOLS cols) + first x chunk
        t0 = ctx.enter_context(
            nc.sbuf_tensor("t0", [P, WCOLS + CHUNK], mybir.dt.float16)
        )
        xts = [t0]
        for d in range(1, NDMA):
            ncols = (min((d + 1) * JPD, J) - d * JPD) * MCOLS
            xts.append(
                ctx.enter_context(nc.sbuf_tensor(f"xt{d}", [P, ncols], mybir.dt.float16))
            )
        out_sb = ctx.enter_context(
            nc.sbuf_tensor("out_sb", [8, MCOLS], mybir.dt.float32)
        )
        acc = ctx.enter_context(nc.psum_tensor("acc", [8, MCOLS], mybir.dt.float32))

        dsems = [ctx.enter_context(nc.semaphore(f"d{d}")) for d in range(NDMA)]
        pe_sem = ctx.enter_context(nc.semaphore("pe"))
        dve_sem = ctx.enter_context(nc.semaphore("dve"))
        osem = ctx.enter_context(nc.semaphore("o"))
        all_sems = dsems + [pe_sem, dve_sem, osem]

        block = ctx.enter_context(nc.Block())

        @block.sync
        def _(sync):
            sync.dma_start(t0[:], xw_in[:, 0 : WCOLS + CHUNK]).then_inc(dsems[0], 16)
            for d in range(1, NDMA):
                ncols = xts[d].shape[1]
                sync.dma_start(
                    xts[d][:], xw_in[:, WCOLS + d * CHUNK : WCOLS + d * CHUNK + ncols]
                ).then_inc(dsems[d], 16)
            sync.wait_ge(dve_sem, 1)
            sync.dma_start(out[:], out_sb[:]).then_inc(osem, 16)

        @block.tensor
        def _(tensor):
            for j in range(J):
                d, jj = divmod(j, JPD)
                if jj == 0:
                    tensor.wait_ge(dsems[d], 16)
                xoff = (WCOLS if d == 0 else 0) + jj * MCOLS
                mm = tensor.matmul(
                    acc[:],
                    t0[:, 8 * j : 8 * j + 8],
                    xts[d][:, xoff : xoff + MCOLS],
                    start=(j == 0),
                    stop=(j == J - 1),
                )
            mm.then_inc(pe_sem, 1)

        @block.vector
        def _(vector):
            # psum rows 0:4 = wh products, rows 4:8 = wl products;
            # cols 0:256 = xh, 256:512 = xr.  DVE cannot read PSUM at a
            # partition offset of 4, so ship the whole [8, 512] block and
            # let the host combine xh*wh + xr*wh + xh*wl.
            vector.wait_ge(pe_sem, 1)
            vector.tensor_copy(out_sb[:], acc[:]).then_inc(dve_sem, 1)

        @block.gpsimd
        def _(gpsimd):
            # reset all sems to 0 after everything finished so a cached
            # NEFF can be re-executed (sem state persists across runs)
            gpsimd.wait_ge(osem, 16)
            for s in all_sems:
                gpsimd.sem_clear(s)

    return nc


def _pack(x, W1, W2):
    xf = np.ascontiguousarray(x, dtype=np.float32).reshape(B, D)
    xh = xf.astype(np.float16)
    xr = (xf - xh.astype(np.float32)).astype(np.float16)

    w4 = np.concatenate(
        [np.asarray(W1, np.float32), np.asarray(W2, np.float32)], axis=1
    )  # [D, 4]
    wh = w4.astype(np.float16)
    wl = (w4 - wh.astype(np.float32)).astype(np.float16)

    xw = np.empty((NCORES, P, TOT), dtype=np.float16)
    # W part: col 8j + h*4 + c  =  w4_part[h][k*DS + j*P + p, c]
    wst = np.stack([wh, wl])  # [2, D, 4]
    xw[:, :, :WCOLS] = (
        wst.reshape(2, NCORES, J, P, 4)
        .transpose(1, 3, 2, 0, 4)
        .reshape(NCORES, P, WCOLS)
    )
    # x part: col WCOLS + j*512 + h*256 + b  =  x_part[h][b, k*DS + j*P + p]
    xst = np.stack([xh, xr])  # [2, B, D]
    xw[:, :, WCOLS:] = (
        xst.reshape(2, B, NCORES, J, P)
        .transpose(2, 4, 3, 0, 1)
        .reshape(NCORES, P, J * MCOLS)
    )
    return xw


def kernel(x, W1, b1, W2, b2, trace=False):
    if "nc" not in _CACHE:
        _CACHE["nc"] = _build()
    nc = _CACHE["nc"]

    xw = _pack(x, W1, W2)
    in_maps = [{"xw": xw[k]} for k in range(NCORES)]
    res = run_bass_kernel_spmd(nc, in_maps, list(range(NCORES)), trace=trace)
    _CACHE["last_results"] = res

    logits4 = np.zeros((4, B), dtype=np.float32)
    for k in range(NCORES):
        r = res.results[k]["partial"]  # [8, 512]
        logits4 += r[0:4, 0:B] + r[0:4, B : 2 * B] + r[4:8, 0:B]

    t_out = logits4[0:2].T + np.asarray(b1, np.float32)  # [256, 2]
    f_out = logits4[2:4].T + np.asarray(b2, np.float32)  # [256, 2]
    m = t_out.max(axis=1, keepdims=True)
    e = np.exp(t_out - m)
    conf = (e / e.sum(axis=1, keepdims=True)).max(axis=1)
    blended = 0.7 * t_out + 0.3 * f_out
    out = np.where((conf > THRESHOLD)[:, None], t_out, blended)
    return out.astype(np.float32)


# revision 10
# speedup vs baseline: 1.4179x; 1.1037x over previous
"""Trainium2 Bass kernel for nn_CascadingSystem (confidence-gated 2-expert blend).

Computation (reference):
    xf = x.reshape(256, 150528)
    t_out = xf @ W1 + b1            # [256, 2]
    f_out = xf @ W2 + b2            # [256, 2]
    conf  = max(softmax(t_out, 1), 1)
    out   = where(conf > 0.95, t_out, 0.7*t_out + 0.3*f_out)

Strategy (memory-bound; reading x dominates; ~358 GB/s HBM per core):
  - Shard the feature dim D=150528 across 8 cores (18816 each). Every core
    streams its d-slice of ALL 256 samples once from HBM and computes the
    partial [4, 256] logits (4 = W1c0, W1c1, W2c0, W2c1) on the tensor
    engine, 147 accumulating matmul chunks of K=128.
  - Precision/bandwidth: fp32 matmuls are 4 cyc/row (PE-bound) and fp32
    data is 4 B/elem (54us stream). Instead decompose on the host
        x = xh(fp16) + xr,   xr8 = fp8_e4m3(xr * 2^12)
        W = wh(fp16) + wl(fp16),  w8 = fp8_e4m3(W * 2^9)
        logits = xh*wh + xh*wl + (xr8*w8) / 2^21
    3 B/elem -> ~41us stream; PE does 2 fp16/fp8 matmuls (1 cyc/row) per
    chunk => ~35us, under the DMA roofline. Max logit error ~1.2e-4; the
    conf>0.95 gate's closest sample sits 7.2e-4 from the threshold and
    conf error is ~1.9e-5 (38x margin). Verified against the fp64
    reference on the real seed-0 inputs.
  - Host pre-packs both streams into PE-ready layout (feature dim on
    partitions, batch on the moving dim), so device DMAs are contiguous.
  - Raw Bass (no TileContext): explicit per-DMA semaphores; the PE chases
    the input DMAs chunk-by-chunk. The fp16 stream is issued by the sync
    engine and the fp8 stream by the scalar engine (both HWDGE) so
    descriptor-issue time is split across two engines.
  - Host sums the 8 partial tensors and applies the tiny
    bias/softmax/threshold/blend epilogue on [256, 4] floats.
"""

from contextlib import ExitStack

import ml_dtypes
import numpy as np

import concourse.bass as bass
import concourse.mybir as mybir
from concourse.bass_utils import run_bass_kernel_spmd

NCORES = 8
B = 256            # batch (matmul moving dim)
D = 150528         # 3*224*224
DS = D // NCORES   # 18816 features per core
P = 128            # partitions / contraction tile
J = DS // P        # 147 matmul chunks per core
NDMA = 21          # input DMA chunks per core (per stream)
JPD = -(-J // NDMA)  # matmul chunks per DMA chunk
W16C = 8 * J       # fp16 weight cols (wh|wl, 4 each, per chunk)
W8C = 4 * J        # fp8 weight cols (4 per chunk)
C16 = JPD * B      # fp16 x cols per full DMA chunk
C8 = JPD * B       # fp8 x cols per full DMA chunk
T16 = W16C + J * B
T8 = W8C + J * B
XS = 2.0 ** 12     # fp8 residual scale
WS = 2.0 ** 9      # fp8 weight scale
THRESHOLD = 0.95

_CACHE = {}


def _build():
    nc = bass.Bass()
    x16_in = nc.declare_dram_parameter("x16", [P, T16], mybir.dt.float16, isOutput=False)
    x8_in = nc.declare_dram_parameter("x8", [P, T8], mybir.dt.float8e4, isOutput=False)
    out = nc.declare_dram_parameter(
        "partial", [8, 2 * B], mybir.dt.float32, isOutput=True
    )

    with ExitStack() as ctx:
        # chunk 0 of each stream carries that stream's W columns
        t16 = [
            ctx.enter_context(
                nc.sbuf_tensor("t16_0", [P, W16C + C16], mybir.dt.float16)
            )
        ]
        t8 = [
            ctx.enter_context(nc.sbuf_tensor("t8_0", [P, W8C + C8], mybir.dt.float8e4))
        ]
        for d in range(1, NDMA):
            ncols = (min((d + 1) * JPD, J) - d * JPD) * B
            t16.append(
                ctx.enter_context(
                    nc.sbuf_tensor(f"t16_{d}", [P, ncols], mybir.dt.float16)
                )
            )
            t8.append(
                ctx.enter_context(
                    nc.sbuf_tensor(f"t8_{d}", [P, ncols], mybir.dt.float8e4)
                )
            )
        out_sb = ctx.enter_context(
            nc.sbuf_tensor("out_sb", [8, 2 * B], mybir.dt.float32)
        )
        acc16 = ctx.enter_context(nc.psum_tensor("acc16", [8, B], mybir.dt.float32))
        acc8 = ctx.enter_context(nc.psum_tensor("acc8", [4, B], mybir.dt.float32))

        s16 = [ctx.enter_context(nc.semaphore(f"s16_{d}")) for d in range(NDMA)]
        s8 = [ctx.enter_context(nc.semaphore(f"s8_{d}")) for d in range(NDMA)]
        pe_sem = ctx.enter_context(nc.semaphore("pe"))
        dve_sem = ctx.enter_context(nc.semaphore("dve"))
        osem = ctx.enter_context(nc.semaphore("o"))
        all_sems = s16 + s8 + [pe_sem, dve_sem, osem]

        block = ctx.enter_context(nc.Block())

        @block.sync
        def _(sync):
            sync.dma_start(t16[0][:], x16_in[:, 0 : W16C + C16]).then_inc(s16[0], 16)
            for d in range(1, NDMA):
                ncols = t16[d].shape[1]
                sync.dma_start(
                    t16[d][:], x16_in[:, W16C + d * C16 : W16C + d * C16 + ncols]
                ).then_inc(s16[d], 16)

        @block.scalar
        def _(scalar):
            scalar.dma_start(t8[0][:], x8_in[:, 0 : W8C + C8]).then_inc(s8[0], 16)
            for d in range(1, NDMA):
                ncols = t8[d].shape[1]
                scalar.dma_start(
                    t8[d][:], x8_in[:, W8C + d * C8 : W8C + d * C8 + ncols]
                ).then_inc(s8[d], 16)
            scalar.wait_ge(dve_sem, 1)
            scalar.dma_start(out[:], out_sb[:]).then_inc(osem, 16)

        @block.tensor
        def _(tensor):
            for j in range(J):
                d, jj = divmod(j, JPD)
                if jj == 0:
                    tensor.wait_ge(s16[d], 16)
                    tensor.wait_ge(s8[d], 16)
                o16 = (W16C if d == 0 else 0) + jj * B
                o8 = (W8C if d == 0 else 0) + jj * B
                tensor.matmul(
                    acc16[:],
                    t16[0][:, 8 * j : 8 * j + 8],
                    t16[d][:, o16 : o16 + B],
                    start=(j == 0),
                    stop=(j == J - 1),
                )
                mm = tensor.matmul(
                    acc8[:],
                    t8[0][:, 4 * j : 4 * j + 4],
                    t8[d][:, o8 : o8 + B],
                    start=(j == 0),
                    stop=(j == J - 1),
                )
            mm.then_inc(pe_sem, 1)

        @block.vector
        def _(vector):
            # out_sb cols 0:256 = fp16 psum [8, 256]; cols 256:512 rows 0:4
            # = fp8 residual psum [4, 256] (scaled by XS*WS).
            vector.wait_ge(pe_sem, 1)
            vector.tensor_copy(out_sb[:, 0:B], acc16[:])
            vector.tensor_copy(out_sb[0:4, B : 2 * B], acc8[:]).then_inc(dve_sem, 1)

        @block.gpsimd
        def _(gpsimd):
            # reset all sems to 0 after everything finished so a cached
            # NEFF can be re-executed (sem state persists across runs)
            gpsimd.wait_ge(osem, 16)
            for s in all_sems:
                gpsimd.sem_clear(s)

    return nc


def _pack(x, W1, W2):
    xf = np.ascontiguousarray(x, dtype=np.float32).reshape(B, D)
    xh = xf.astype(np.float16)
    xr8 = ((xf - xh.astype(np.float32)) * np.float32(XS)).astype(ml_dtypes.float8_e4m3)

    w4 = np.concatenate(
        [np.asarray(W1, np.float32), np.asarray(W2, np.float32)], axis=1
    )  # [D, 4]
    wh = w4.astype(np.float16)
    wl = (w4 - wh.astype(np.float32)).astype(np.float16)
    w8 = (w4 * np.float32(WS)).astype(ml_dtypes.float8_e4m3)

    xw16 = np.empty((NCORES, P, T16), dtype=np.float16)
    # fp16 W part: col 8j + h*4 + c = (wh,wl)[h][k*DS + j*P + p, c]
    wst = np.stack([wh, wl])  # [2, D, 4]
    xw16[:, :, :W16C] = (
        wst.reshape(2, NCORES, J, P, 4)
        .transpose(1, 3, 2, 0, 4)
        .reshape(NCORES, P, W16C)
    )
    # fp16 x part: col W16C + j*B + b = xh[b, k*DS + j*P + p]
    xw16[:, :, W16C:] = (
        xh.reshape(B, NCORES, J, P).transpose(1, 3, 2, 0).reshape(NCORES, P, J * B)
    )

    xw8 = np.empty((NCORES, P, T8), dtype=ml_dtypes.float8_e4m3)
    xw8[:, :, :W8C] = (
        w8.reshape(NCORES, J, P, 4).transpose(0, 2, 1, 3).reshape(NCORES, P, W8C)
    )
    xw8[:, :, W8C:] = (
        xr8.reshape(B, NCORES, J, P).transpose(1, 3, 2, 0).reshape(NCORES, P, J * B)
    )
    return xw16, xw8


def kernel(x, W1, b1, W2, b2, trace=False):
    if "nc" not in _CACHE:
        _CACHE["nc"] = _build()
    nc = _CACHE["nc"]

    xw16, xw8 = _pack(x, W1, W2)
    in_maps = [{"x16": xw16[k], "x8": xw8[k]} for k in range(NCORES)]
    res = run_bass_kernel_spmd(nc, in_maps, list(range(NCORES)), trace=trace)
    _CACHE["last_results"] = res

    logits4 = np.zeros((4, B), dtype=np.float64)
    for k in range(NCORES):
        r = res.results[k]["partial"]  # [8, 512]
        logits4 += r[0:4, 0:B] + r[4:8, 0:B]
        logits4 += r[0:4, B : 2 * B].astype(np.float64) / (XS * WS)
    logits4 = logits4.astype(np.float32)

    t_out = logits4[0:2].T + np.asarray(b1, np.float32)  # [256, 2]
    f_out = logits4[2:4].T + np.asarray(b2, np.float32)  # [256, 2]
    m = t_out.max(axis=1, keepdims=True)
    e = np.exp(t_out - m)
    conf = (e / e.sum(axis=1, keepdims=True)).max(axis=1)
    blended = 0.7 * t_out + 0.3 * f_out
    out = np.where((conf > THRESHOLD)[:, None], t_out, blended)
    return out.astype(np.float32)


# revision 11
# speedup vs baseline: 1.4631x; 1.0319x over previous
"""Trainium2 Bass kernel for nn_CascadingSystem (confidence-gated 2-expert blend).

Computation (reference):
    xf = x.reshape(256, 150528)
    t_out = xf @ W1 + b1            # [256, 2]
    f_out = xf @ W2 + b2            # [256, 2]
    conf  = max(softmax(t_out, 1), 1)
    out   = where(conf > 0.95, t_out, 0.7*t_out + 0.3*f_out)

Strategy (memory-bound; reading x dominates; ~358 GB/s HBM per core):
  - Shard the feature dim D=150528 across 8 cores (18816 each). Every core
    streams its d-slice of ALL 256 samples once from HBM and computes the
    partial [4, 256] logits (4 = W1c0, W1c1, W2c0, W2c1) on the tensor
    engine, 147 accumulating matmul chunks of K=128.
  - Precision/bandwidth: fp32 matmuls are 4 cyc/row (PE-bound) and fp32
    data is 4 B/elem (54us stream). Instead decompose on the host
        x = xh(fp16) + xr,   xr8 = fp8_e4m3(xr * 2^12)
        W = wh(fp16) + wl(fp16),  w8 = fp8_e4m3(W * 2^9)
        logits = xh*wh + xh*wl + (xr8*w8) / 2^21
    3 B/elem -> ~41us stream; PE does 2 fp16/fp8 matmuls (1 cyc/row) per
    chunk => ~33us, under the DMA roofline. Max logit error ~1.2e-4; the
    conf>0.95 gate's closest sample sits 7.2e-4 from the threshold and
    conf error is ~1.9e-5 (38x margin). Verified against the fp64
    reference on the real seed-0 inputs.
  - Host pre-packs both streams into PE-ready layout (feature dim on
    partitions, batch on the moving dim), so device DMAs are contiguous.
  - Raw Bass (no TileContext): explicit per-DMA semaphores; the PE chases
    the input DMAs chunk-by-chunk. The fp16 stream is issued by the sync
    engine and the fp8 stream by the scalar engine (both HWDGE) so
    descriptor-issue time is split across two engines. Chunk sizes ramp
    up (1 -> 18 matmul chunks) so the PE starts as early as possible.
  - Host sums the 8 partial tensors and applies the tiny
    bias/softmax/threshold/blend epilogue on [256, 4] floats.
"""

from contextlib import ExitStack

import ml_dtypes
import numpy as np

import concourse.bass as bass
import concourse.mybir as mybir
from concourse.bass_utils import run_bass_kernel_spmd

NCORES = 8
B = 256            # batch (matmul moving dim)
D = 150528         # 3*224*224
DS = D // NCORES   # 18816 features per core
P = 128            # partitions / contraction tile
J = DS // P        # 147 matmul chunks per core
# j-chunks per DMA: small first chunks let the PE start early
SIZES = [1, 2, 4, 6, 8, 10, 12, 14, 16, 18, 18, 14, 12, 12]
assert sum(SIZES) == J
STARTS = [sum(SIZES[:i]) for i in range(len(SIZES))]
NDMA = len(SIZES)
W16C = 8 * J       # fp16 weight cols (wh|wl, 4 each, per chunk)
W8C = 4 * J        # fp8 weight cols (4 per chunk)
T16 = W16C + J * B
T8 = W8C + J * B
XS = 2.0 ** 12     # fp8 residual scale
WS = 2.0 ** 9      # fp8 weight scale
THRESHOLD = 0.95

_CACHE = {}


def _build():
    nc = bass.Bass()
    x16_in = nc.declare_dram_parameter("x16", [P, T16], mybir.dt.float16, isOutput=False)
    x8_in = nc.declare_dram_parameter("x8", [P, T8], mybir.dt.float8e4, isOutput=False)
    out = nc.declare_dram_parameter(
        "partial", [8, 2 * B], mybir.dt.float32, isOutput=True
    )

    with ExitStack() as ctx:
        # chunk 0 of each stream carries that stream's W columns
        t16 = []
        t8 = []
        for d in range(NDMA):
            n16 = SIZES[d] * B + (W16C if d == 0 else 0)
            n8 = SIZES[d] * B + (W8C if d == 0 else 0)
            t16.append(
                ctx.enter_context(nc.sbuf_tensor(f"t16_{d}", [P, n16], mybir.dt.float16))
            )
            t8.append(
                ctx.enter_context(nc.sbuf_tensor(f"t8_{d}", [P, n8], mybir.dt.float8e4))
            )
        out_sb = ctx.enter_context(
            nc.sbuf_tensor("out_sb", [8, 2 * B], mybir.dt.float32)
        )
        acc16 = ctx.enter_context(nc.psum_tensor("acc16", [8, B], mybir.dt.float32))
        acc8 = ctx.enter_context(nc.psum_tensor("acc8", [4, B], mybir.dt.float32))

        s16 = [ctx.enter_context(nc.semaphore(f"s16_{d}")) for d in range(NDMA)]
        s8 = [ctx.enter_context(nc.semaphore(f"s8_{d}")) for d in range(NDMA)]
        pe_sem = ctx.enter_context(nc.semaphore("pe"))
        dve_sem = ctx.enter_context(nc.semaphore("dve"))
        osem = ctx.enter_context(nc.semaphore("o"))
        all_sems = s16 + s8 + [pe_sem, dve_sem, osem]
        sem_nums = sorted(s.num for s in all_sems)
        assert sem_nums == list(range(sem_nums[0], sem_nums[-1] + 1))
        sem_range = range(sem_nums[0], sem_nums[-1] + 1)

        block = ctx.enter_context(nc.Block())

        @block.sync
        def _(sync):
            for d in range(NDMA):
                c0 = STARTS[d] * B + (0 if d == 0 else W16C)
                sync.dma_start(
                    t16[d][:], x16_in[:, c0 : c0 + t16[d].shape[1]]
                ).then_inc(s16[d], 16)

        @block.scalar
        def _(scalar):
            for d in range(NDMA):
                c0 = STARTS[d] * B + (0 if d == 0 else W8C)
                scalar.dma_start(
                    t8[d][:], x8_in[:, c0 : c0 + t8[d].shape[1]]
                ).then_inc(s8[d], 16)
            scalar.wait_ge(dve_sem, 1)
            scalar.dma_start(out[:], out_sb[:]).then_inc(osem, 16)

        @block.tensor
        def _(tensor):
            for d in range(NDMA):
                tensor.wait_ge(s16[d], 16)
                tensor.wait_ge(s8[d], 16)
                for jj in range(SIZES[d]):
                    j = STARTS[d] + jj
                    o16 = (W16C if d == 0 else 0) + jj * B
                    o8 = (W8C if d == 0 else 0) + jj * B
                    tensor.matmul(
                        acc16[:],
                        t16[0][:, 8 * j : 8 * j + 8],
                        t16[d][:, o16 : o16 + B],
                        start=(j == 0),
                        stop=(j == J - 1),
                    )
                    mm = tensor.matmul(
                        acc8[:],
                        t8[0][:, 4 * j : 4 * j + 4],
                        t8[d][:, o8 : o8 + B],
                        start=(j == 0),
                        stop=(j == J - 1),
                    )
            mm.then_inc(pe_sem, 1)

        @block.vector
        def _(vector):
            # out_sb cols 0:256 = fp16 psum [8, 256]; cols 256:512 rows 0:4
            # = fp8 residual psum [4, 256] (scaled by XS*WS).
            vector.wait_ge(pe_sem, 1)
            vector.tensor_copy(out_sb[:, 0:B], acc16[:])
            vector.tensor_copy(out_sb[0:4, B : 2 * B], acc8[:]).then_inc(dve_sem, 1)

        @block.gpsimd
        def _(gpsimd):
            # reset all sems to 0 after everything finished so a cached
            # NEFF can be re-executed (sem state persists across runs)
            gpsimd.wait_ge(osem, 16)
            gpsimd.sem_clear(sem_range)

    return nc


def _pack(x, W1, W2):
    xf = np.ascontiguousarray(x, dtype=np.float32).reshape(B, D)
    xh = xf.astype(np.float16)
    xr8 = ((xf - xh.astype(np.float32)) * np.float32(XS)).astype(ml_dtypes.float8_e4m3)

    w4 = np.concatenate(
        [np.asarray(W1, np.float32), np.asarray(W2, np.float32)], axis=1
    )  # [D, 4]
    wh = w4.astype(np.float16)
    wl = (w4 - wh.astype(np.float32)).astype(np.float16)
    w8 = (w4 * np.float32(WS)).astype(ml_dtypes.float8_e4m3)

    xw16 = np.empty((NCORES, P, T16), dtype=np.float16)
    # fp16 W part: col 8j + h*4 + c = (wh,wl)[h][k*DS + j*P + p, c]
    wst = np.stack([wh, wl])  # [2, D, 4]
    xw16[:, :, :W16C] = (
        wst.reshape(2, NCORES, J, P, 4)
        .transpose(1, 3, 2, 0, 4)
        .reshape(NCORES, P, W16C)
    )
    # fp16 x part: col W16C + j*B + b = xh[b, k*DS + j*P + p]
    xw16[:, :, W16C:] = (
        xh.reshape(B, NCORES, J, P).transpose(1, 3, 2, 0).reshape(NCORES, P, J * B)
    )

    xw8 = np.empty((NCORES, P, T8), dtype=ml_dtypes.float8_e4m3)
    xw8[:, :, :W8C] = (
        w8.reshape(NCORES, J, P, 4).transpose(0, 2, 1, 3).reshape(NCORES, P, W8C)
    )
    xw8[:, :, W8C:] = (
        xr8.reshape(B, NCORES, J, P).transpose(1, 3, 2, 0).reshape(NCORES, P, J * B)
    )
    return xw16, xw8


def kernel(x, W1, b1, W2, b2, trace=False):
    if "nc" not in _CACHE:
        _CACHE["nc"] = _build()
    nc = _CACHE["nc"]

    xw16, xw8 = _pack(x, W1, W2)
    in_maps = [{"x16": xw16[k], "x8": xw8[k]} for k in range(NCORES)]
    res = run_bass_kernel_spmd(nc, in_maps, list(range(NCORES)), trace=trace)
    _CACHE["last_results"] = res

    logits4 = np.zeros((4, B), dtype=np.float64)
    for k in range(NCORES):
        r = res.results[k]["partial"]  # [8, 512]
        logits4 += r[0:4, 0:B] + r[4:8, 0:B]
        logits4 += r[0:4, B : 2 * B].astype(np.float64) / (XS * WS)
    logits4 = logits4.astype(np.float32)

    t_out = logits4[0:2].T + np.asarray(b1, np.float32)  # [256, 2]
    f_out = logits4[2:4].T + np.asarray(b2, np.float32)  # [256, 2]
    m = t_out.max(axis=1, keepdims=True)
    e = np.exp(t_out - m)
    conf = (e / e.sum(axis=1, keepdims=True)).max(axis=1)
    blended = 0.7 * t_out + 0.3 * f_out
    out = np.where((conf > THRESHOLD)[:, None], t_out, blended)
    return out.astype(np.float32)
